# revision 35
# baseline (speedup 1.0000x reference)
"""BPMLL loss kernel for Trainium2, 8-core data parallel (raw bacc).

Reference computation (B=128, L=1024):
    y[b,i]     = target[b,i] == 1
    truth      = y[:,:,None] & ~y[:,None,:]
    inner[b]   = sum_{i,j} truth * exp(x[b,j] - x[b,i])
    length[b]  = n_pos[b] * n_neg[b]
    out        = sum_b inner[b] / length[b]

The O(L^2) pairwise sum factorizes:
    inner[b] = (sum_{j: ~y} exp(x[b,j])) * (sum_{i: y} exp(-x[b,i]))

Sharding: pure data parallel. The batch is split across 8 cores (16 samples
each); every core runs the same NEFF on its own slice, producing its 16
per-sample losses, and the host sums the 8x16 results (the all-reduce /
gather step of the data-parallel scheme).

Per-core layout: the [16, 1024] slice is viewed as [128, 128] (8 SBUF
partition rows per sample).  All per-core inputs travel as ONE packed
[128, 704] uint8 DMA (per partition row: 512B of x as f32, 128B of 0/1 mask,
64B of a constant segment matrix) — a single HWDGE transfer eats the fixed
~1.7us DMA pipeline latency once.

Masking folds into the exp argument (t1 = x - 100*y):
    exp(t1)        = (1-y)*exp(x)  + O(e^-95)    row-sums via ACT accumulate
    exp(-t1 - 100) = y*exp(-x)     + O(e^-95)
A [128,16] segment matmul reduces the 8 rows of each sample; it is split in
two (different PSUM banks) so the n_pos-dependent tail overlaps exp2.  The
tail computes loss = S_neg*S_pos / (n_pos*(L-n_pos)) per sample with a
fused sign trick (4 DVE ops).

Raw bacc (no TileContext) with manual semaphores saves ~1.3us of framework
drain/barrier overhead; a dummy activation right after the DMA trigger
hoists the ACT Exp table load into the DMA wait.
"""

import os
import sys

import numpy as np

if "/opt/trn_rl_repo" not in sys.path:
    sys.path.insert(0, "/opt/trn_rl_repo")

from contextlib import ExitStack

import concourse.bass as bass  # noqa: F401
from concourse import bacc, mybir
from concourse.bass_utils import run_bass_kernel_spmd

B, L = 128, 1024
NCORES = 8
BS = B // NCORES            # 16 samples per core
P = 128                     # SBUF partitions
F = (BS * L) // P           # 128 free elements per partition row
RPS = P // BS               # 8 partition rows per sample
MASK_BIG = 100.0            # exp(-95) ~ 5e-42: kills masked terms in f32 sums
BLOB = F * 4 + F + BS * 4   # 704 bytes per partition row

_cached_nc = None


def _ensure_ntff_hook():
    """Provide antenv.axon_hooks if the image lacks it, so trace=True /
    BASS_TRACE=1 profiling works instead of crashing on import."""
    import types

    try:
        from antenv.axon_hooks import get_axon_ntff_profile_hook  # noqa: F401

        return
    except ImportError:
        pass
    try:
        import antenv
    except ImportError:
        return
    mod = types.ModuleType("antenv.axon_hooks")
    mod._hook = None

    def set_axon_ntff_profile_hook(h):
        mod._hook = h

    def get_axon_ntff_profile_hook():
        return mod._hook

    mod.set_axon_ntff_profile_hook = set_axon_ntff_profile_hook
    mod.get_axon_ntff_profile_hook = get_axon_ntff_profile_hook
    sys.modules["antenv.axon_hooks"] = mod
    antenv.axon_hooks = mod
    try:
        from trn_agent_boot.trn_boot import _ntff_profile_via_ctypes

        hook = _ntff_profile_via_ctypes("/opt/axon/libaxon_pjrt.so")
        if hook is not None:
            mod._hook = hook
    except Exception:
        pass


_ensure_ntff_hook()


def _build_module():
    nc = bacc.Bacc(
        "TRN2",
        target_bir_lowering=False,
        debug=False,
        num_devices=NCORES,
    )
    blob_d = nc.dram_tensor(
        "blob", [P, BLOB], mybir.dt.uint8, kind="ExternalInput"
    ).ap()
    out_d = nc.dram_tensor("out", [BS, 1], mybir.dt.float32, kind="ExternalOutput").ap()

    with ExitStack() as ctx:
        sb = lambda name, shape, dt=mybir.dt.float32: ctx.enter_context(  # noqa: E731
            nc.sbuf_tensor(name, shape, dt)
        ).ap()
        sem = lambda name: ctx.enter_context(nc.semaphore(name))  # noqa: E731

        blob = sb("blob_t", [P, BLOB], mybir.dt.uint8)
        t1 = sb("t1", [P, F])
        e1 = sb("e1", [P, F])
        e2 = sb("e2", [P, F])
        stats = sb("stats", [P, 3])
        nbig = sb("nbig", [P, 1])
        junk = sb("junk", [1, 1])
        nlen = sb("nlen", [BS, 1])
        rlen = sb("rlen", [BS, 1])
        u = sb("u", [BS, 1])
        loss = sb("loss", [BS, 1])
        ps_a = ctx.enter_context(nc.psum_tensor("ps_a", [BS, 2], mybir.dt.float32)).ap()
        ps_b = ctx.enter_context(nc.psum_tensor("ps_b", [BS, 1], mybir.dt.float32)).ap()

        x_t = blob[:, 0 : F * 4].bitcast(mybir.dt.float32)
        y_t = blob[:, F * 4 : F * 4 + F]
        seg_t = blob[:, F * 4 + F : BLOB].bitcast(mybir.dt.float32)

        s_in = sem("s_in")
        s_g = sem("s_g")
        s_v = sem("s_v")
        s_s = sem("s_s")
        s_t = sem("s_t")
        s_c = sem("s_c")
        s_v2 = sem("s_v2")
        s_out = sem("s_out")

        # GpSimd: exp2's bias constant (hoisted pre-barrier; GpSimd boots
        # early, and the dummy exp below reuses it as a harmless input).
        memset_bi = nc.gpsimd.memset(nbig[:], -MASK_BIG).then_inc(s_g, 1)

        # Scalar: input DMA trigger first (no deps), then the dummy exp pulls
        # the ACT Exp table in during the DMA wait.  All hoisted ahead of
        # the framework preamble barrier below, so none may touch the
        # framework const APs (input/bias come from nbig for that reason;
        # exp(-200) in a junk tile is harmless).
        dma_in_bi = nc.scalar.dma_start(blob[:], blob_d).then_inc(s_in, 16)
        wait_bi = nc.scalar.wait_ge(s_g, 1)
        dummy_bi = nc.scalar.activation(
            junk[:], nbig[0:1, :], mybir.ActivationFunctionType.Exp, bias=nbig[0:1, :]
        )

        # Vector: t1 = x - 100*y ; npos per row (independent, pipelined).
        nc.vector.wait_ge(s_in, 16)
        nc.vector.scalar_tensor_tensor(
            t1[:],
            y_t[:],
            -MASK_BIG,
            x_t[:],
            op0=mybir.AluOpType.mult,
            op1=mybir.AluOpType.add,
        ).then_inc(s_v, 1)
        nc.vector.reduce_sum(
            stats[:, 1:2], y_t[:], axis=mybir.AxisListType.X
        ).then_inc(s_v, 1)

        # Scalar: masked exp row sums via the ACT accumulator.
        nc.scalar.wait_ge(s_v, 1)
        nc.scalar.activation(
            e1[:],
            t1[:],
            mybir.ActivationFunctionType.Exp,
            accum_out=stats[:, 0:1],
        ).then_inc(s_s, 1)
        nc.scalar.wait_ge(s_g, 1)
        nc.scalar.activation(
            e2[:],
            t1[:],
            mybir.ActivationFunctionType.Exp,
            bias=nbig[:],
            scale=-1.0,
            accum_out=stats[:, 2:3],
        ).then_inc(s_s, 1)

        # Tensor: segment reduce, two PSUM banks so the PE write of ps_b can
        # overlap the DVE reads of ps_a.
        nc.tensor.wait_ge(s_in, 16)
        nc.tensor.wait_ge(s_v, 2)
        nc.tensor.wait_ge(s_s, 1)
        nc.tensor.matmul(ps_a[:], seg_t[:], stats[:, 0:2]).then_inc(s_t, 1)
        nc.tensor.wait_ge(s_s, 2)
        nc.tensor.matmul(ps_b[:], seg_t[:], stats[:, 2:3]).then_inc(s_t, 1)

        # Vector tail: loss = S_neg*S_pos / (n_pos*(L-n_pos)) with a sign
        # trick: nlen = (n_pos - L)*n_pos = -length, u = (-S_neg)/nlen,
        # loss = S_pos * u.  DVE has no internal scoreboard: dependent
        # same-engine ops need explicit sem hops (s_c chain).
        nc.vector.wait_ge(s_t, 1)
        nc.vector.tensor_scalar(
            nlen[:],
            ps_a[:, 1:2],
            float(L),
            ps_a[:, 1:2],
            op0=mybir.AluOpType.subtract,
            op1=mybir.AluOpType.mult,
        ).then_inc(s_c, 1)
        nc.vector.wait_ge(s_c, 1)
        nc.vector.reciprocal(rlen[:], nlen[:]).then_inc(s_c, 1)
        nc.vector.wait_ge(s_c, 2)
        nc.vector.scalar_tensor_tensor(
            u[:],
            ps_a[:, 0:1],
            -1.0,
            rlen[:],
            op0=mybir.AluOpType.mult,
            op1=mybir.AluOpType.mult,
        ).then_inc(s_c, 1)
        nc.vector.wait_ge(s_t, 2)
        nc.vector.wait_ge(s_c, 3)
        nc.vector.tensor_mul(loss[:], ps_b[:, 0:1], u[:]).then_inc(s_v2, 1)

        # Sync: write the result out, then drain.
        nc.sync.wait_ge(s_v2, 1)
        nc.sync.dma_start(out_d, loss[:], single_packet=True).then_inc(s_out, 16)
        nc.sync.wait_ge(s_out, 16)

        # Hoist the input DMA (and the ACT-table-loading dummy exp) ahead of
        # the Bass preamble's const-memset barrier: neither touches const
        # APs, so the ~2.4us DMA pipeline latency overlaps the ~6us engine
        # boot + barrier + IRAM fetch preamble instead of following it.
        hoisted = [memset_bi.ins, dma_in_bi.ins, wait_bi.ins, dummy_bi.ins]
        ids = {id(o) for o in hoisted}
        for b in nc.m.functions[0].blocks:
            il = b.instructions
            kept = [i for i in il if id(i) not in ids]
            if len(kept) != len(il):
                il[:] = kept
        b0 = nc.m.functions[0].blocks[0].instructions
        b0[:] = hoisted + b0

    nc.compile()

    # bacc's compile inserts the ACT table load (InstLoadActFuncSet) ahead of
    # the input DMA; its 8KB transfer then contends with the 88KB blob on the
    # shared DMA path.  The table isn't needed until exp1 (~2.5us later), so
    # move it to just after the DMA descriptor.
    b0 = nc.m.functions[0].blocks[0].instructions
    tbl_idx = next(
        (k for k, i in enumerate(b0) if type(i).__name__ == "InstLoadActFuncSet"),
        None,
    )
    dma_idx = next(k for k, i in enumerate(b0) if i is dma_in_bi.ins)
    if tbl_idx is not None and tbl_idx < dma_idx:
        tbl = b0.pop(tbl_idx)
        b0.insert(dma_idx, tbl)  # dma shifted left by the pop -> lands after it

    return nc


def get_module():
    global _cached_nc
    if _cached_nc is None:
        _cached_nc = _build_module()
    return _cached_nc


def _make_seg() -> np.ndarray:
    seg = np.zeros((P, BS), dtype=np.float32)
    seg[np.arange(P), np.arange(P) // RPS] = 1.0
    return seg


def make_in_maps(input: np.ndarray, target: np.ndarray) -> list[dict]:
    x = np.ascontiguousarray(input, dtype=np.float32)
    y = np.ascontiguousarray((target != 0).astype(np.uint8))
    seg8 = _make_seg().view(np.uint8)  # [P, BS*4]
    in_maps = []
    for c in range(NCORES):
        xs8 = x[c * BS : (c + 1) * BS].reshape(P, F).view(np.uint8)  # [P, F*4]
        ys8 = y[c * BS : (c + 1) * BS].reshape(P, F)  # [P, F]
        blob = np.concatenate([xs8, ys8, seg8], axis=1)  # [P, 704] u8
        in_maps.append({"blob": blob})
    return in_maps


def kernel(input: np.ndarray, target: np.ndarray) -> np.ndarray:
    input = np.asarray(input)
    target = np.asarray(target)
    assert input.shape == (B, L) and target.shape == (B, L)
    nc = get_module()
    in_maps = make_in_maps(input, target)
    res = run_bass_kernel_spmd(nc, in_maps, core_ids=list(range(NCORES)))
    losses = np.concatenate([np.asarray(r["out"]).reshape(BS) for r in res.results])
    return np.asarray(losses.sum(), dtype=np.float32)


# revision 36
# speedup vs baseline: 1.1920x; 1.1920x over previous
"""BPMLL loss kernel for Trainium2, 8-core data parallel (raw bacc).

Reference computation (B=128, L=1024):
    y[b,i]     = target[b,i] == 1
    truth      = y[:,:,None] & ~y[:,None,:]
    inner[b]   = sum_{i,j} truth * exp(x[b,j] - x[b,i])
    length[b]  = n_pos[b] * n_neg[b]
    out        = sum_b inner[b] / length[b]

The O(L^2) pairwise sum factorizes:
    inner[b] = (sum_{j: ~y} exp(x[b,j])) * (sum_{i: y} exp(-x[b,i]))

Sharding: pure data parallel. The batch is split across 8 cores (16 samples
each); every core runs the same NEFF on its own slice, producing its 16
per-sample losses, and the host sums the 8x16 results (the all-reduce /
gather step of the data-parallel scheme).

Per-core layout: the [16, 1024] slice is viewed as [128, 128] (8 SBUF
partition rows per sample).  All per-core inputs travel as ONE packed
[128, 704] uint8 DMA (per partition row: 512B of x as f32, 128B of 0/1 mask,
64B of a constant segment matrix) — a single HWDGE transfer eats the fixed
~1.7us DMA pipeline latency once.

Masking folds into the exp argument (t1 = x - 100*y):
    exp(t1)        = (1-y)*exp(x)  + O(e^-95)    row-sums via ACT accumulate
    exp(-t1 - 100) = y*exp(-x)     + O(e^-95)
A [128,16] segment matmul reduces the 8 rows of each sample; it is split in
two (different PSUM banks) so the n_pos-dependent tail overlaps exp2.  The
tail computes loss = S_neg*S_pos / (n_pos*(L-n_pos)) per sample with a
fused sign trick (4 DVE ops).

Raw bacc (no TileContext) with manual semaphores saves ~1.3us of framework
drain/barrier overhead; a dummy activation right after the DMA trigger
hoists the ACT Exp table load into the DMA wait.
"""

import os
import sys

import numpy as np

if "/opt/trn_rl_repo" not in sys.path:
    sys.path.insert(0, "/opt/trn_rl_repo")

from contextlib import ExitStack

import concourse.bass as bass  # noqa: F401
from concourse import bacc, mybir
from concourse.bass_utils import run_bass_kernel_spmd

B, L = 128, 1024
NCORES = 8
BS = B // NCORES            # 16 samples per core
P = 128                     # SBUF partitions
F = (BS * L) // P           # 128 free elements per partition row
RPS = P // BS               # 8 partition rows per sample
MASK_BIG = 100.0            # exp(-95) ~ 5e-42: kills masked terms in f32 sums
BLOB = F * 4 + F + BS * 4   # 704 bytes per partition row

_cached_nc = None


def _ensure_ntff_hook():
    """Provide antenv.axon_hooks if the image lacks it, so trace=True /
    BASS_TRACE=1 profiling works instead of crashing on import."""
    import types

    try:
        from antenv.axon_hooks import get_axon_ntff_profile_hook  # noqa: F401

        return
    except ImportError:
        pass
    try:
        import antenv
    except ImportError:
        return
    mod = types.ModuleType("antenv.axon_hooks")
    mod._hook = None

    def set_axon_ntff_profile_hook(h):
        mod._hook = h

    def get_axon_ntff_profile_hook():
        return mod._hook

    mod.set_axon_ntff_profile_hook = set_axon_ntff_profile_hook
    mod.get_axon_ntff_profile_hook = get_axon_ntff_profile_hook
    sys.modules["antenv.axon_hooks"] = mod
    antenv.axon_hooks = mod
    try:
        from trn_agent_boot.trn_boot import _ntff_profile_via_ctypes

        hook = _ntff_profile_via_ctypes("/opt/axon/libaxon_pjrt.so")
        if hook is not None:
            mod._hook = hook
    except Exception:
        pass


_ensure_ntff_hook()


def _build_module():
    nc = bacc.Bacc(
        "TRN2",
        target_bir_lowering=False,
        debug=False,
        num_devices=NCORES,
    )
    blob_d = nc.dram_tensor(
        "blob", [P, BLOB], mybir.dt.uint8, kind="ExternalInput"
    ).ap()
    out_d = nc.dram_tensor("out", [BS, 1], mybir.dt.float32, kind="ExternalOutput").ap()

    with ExitStack() as ctx:
        sb = lambda name, shape, dt=mybir.dt.float32: ctx.enter_context(  # noqa: E731
            nc.sbuf_tensor(name, shape, dt)
        ).ap()
        sem = lambda name: ctx.enter_context(nc.semaphore(name))  # noqa: E731

        blob = sb("blob_t", [P, BLOB], mybir.dt.uint8)
        t1 = sb("t1", [P, F])
        e1 = sb("e1", [P, F])
        e2 = sb("e2", [P, F])
        stats = sb("stats", [P, 3])
        nbig = sb("nbig", [P, 1])
        junk = sb("junk", [1, 1])
        nlen = sb("nlen", [BS, 1])
        rlen = sb("rlen", [BS, 1])
        u = sb("u", [BS, 1])
        loss = sb("loss", [BS, 1])
        ps_a = ctx.enter_context(nc.psum_tensor("ps_a", [BS, 2], mybir.dt.float32)).ap()
        ps_b = ctx.enter_context(nc.psum_tensor("ps_b", [BS, 1], mybir.dt.float32)).ap()

        x_t = blob[:, 0 : F * 4].bitcast(mybir.dt.float32)
        y_t = blob[:, F * 4 : F * 4 + F]
        seg_t = blob[:, F * 4 + F : BLOB].bitcast(mybir.dt.float32)

        s_in = sem("s_in")
        s_g = sem("s_g")
        s_v = sem("s_v")
        s_s = sem("s_s")
        s_t = sem("s_t")
        s_c = sem("s_c")
        s_v2 = sem("s_v2")
        s_out = sem("s_out")

        # GpSimd: exp2's bias constant (hoisted pre-barrier; GpSimd boots
        # early, and the dummy exp below reuses it as a harmless input).
        memset_bi = nc.gpsimd.memset(nbig[:], -MASK_BIG).then_inc(s_g, 1)

        # Scalar: input DMA trigger first (no deps), then the dummy exp pulls
        # the ACT Exp table in during the DMA wait.  All hoisted ahead of
        # the framework preamble barrier below, so none may touch the
        # framework const APs (input/bias come from nbig for that reason;
        # exp(-200) in a junk tile is harmless).
        dma_in_bi = nc.scalar.dma_start(blob[:], blob_d).then_inc(s_in, 16)
        wait_bi = nc.scalar.wait_ge(s_g, 1)
        dummy_bi = nc.scalar.activation(
            junk[:], nbig[0:1, :], mybir.ActivationFunctionType.Exp, bias=nbig[0:1, :]
        )

        # Vector: t1 = x - 100*y ; npos per row (independent, pipelined).
        nc.vector.wait_ge(s_in, 16)
        nc.vector.scalar_tensor_tensor(
            t1[:],
            y_t[:],
            -MASK_BIG,
            x_t[:],
            op0=mybir.AluOpType.mult,
            op1=mybir.AluOpType.add,
        ).then_inc(s_v, 1)
        nc.vector.reduce_sum(
            stats[:, 1:2], y_t[:], axis=mybir.AxisListType.X
        ).then_inc(s_v, 1)

        # Scalar: masked exp row sums via the ACT accumulator.
        nc.scalar.wait_ge(s_v, 1)
        nc.scalar.activation(
            e1[:],
            t1[:],
            mybir.ActivationFunctionType.Exp,
            accum_out=stats[:, 0:1],
        ).then_inc(s_s, 1)
        nc.scalar.wait_ge(s_g, 1)
        nc.scalar.activation(
            e2[:],
            t1[:],
            mybir.ActivationFunctionType.Exp,
            bias=nbig[:],
            scale=-1.0,
            accum_out=stats[:, 2:3],
        ).then_inc(s_s, 1)

        # Tensor: segment reduce, two PSUM banks so the PE write of ps_b can
        # overlap the DVE reads of ps_a.
        nc.tensor.wait_ge(s_in, 16)
        nc.tensor.wait_ge(s_v, 2)
        nc.tensor.wait_ge(s_s, 1)
        nc.tensor.matmul(ps_a[:], seg_t[:], stats[:, 0:2]).then_inc(s_t, 1)
        nc.tensor.wait_ge(s_s, 2)
        nc.tensor.matmul(ps_b[:], seg_t[:], stats[:, 2:3]).then_inc(s_t, 1)

        # Vector tail: loss = S_neg*S_pos / (n_pos*(L-n_pos)) with a sign
        # trick: nlen = (n_pos - L)*n_pos = -length, u = (-S_neg)/nlen,
        # loss = S_pos * u.  DVE has no internal scoreboard: dependent
        # same-engine ops need explicit sem hops (s_c chain).
        nc.vector.wait_ge(s_t, 1)
        nc.vector.tensor_scalar(
            nlen[:],
            ps_a[:, 1:2],
            float(L),
            ps_a[:, 1:2],
            op0=mybir.AluOpType.subtract,
            op1=mybir.AluOpType.mult,
        ).then_inc(s_c, 1)
        nc.vector.wait_ge(s_c, 1)
        nc.vector.reciprocal(rlen[:], nlen[:]).then_inc(s_c, 1)
        nc.vector.wait_ge(s_c, 2)
        nc.vector.scalar_tensor_tensor(
            u[:],
            ps_a[:, 0:1],
            -1.0,
            rlen[:],
            op0=mybir.AluOpType.mult,
            op1=mybir.AluOpType.mult,
        ).then_inc(s_c, 1)
        nc.vector.wait_ge(s_t, 2)
        nc.vector.wait_ge(s_c, 3)
        nc.vector.tensor_mul(loss[:], ps_b[:, 0:1], u[:]).then_inc(s_v2, 1)

        # Sync: write the result out, then drain.
        nc.sync.wait_ge(s_v2, 1)
        nc.sync.dma_start(out_d, loss[:], single_packet=True).then_inc(s_out, 16)
        nc.sync.wait_ge(s_out, 16)

        # Hoist the input DMA (and the ACT-table-loading dummy exp) ahead of
        # the Bass preamble's const-memset barrier: neither touches const
        # APs, so the ~2.4us DMA pipeline latency overlaps the ~6us engine
        # boot + barrier + IRAM fetch preamble instead of following it.
        hoisted = [memset_bi.ins, dma_in_bi.ins, wait_bi.ins, dummy_bi.ins]
        ids = {id(o) for o in hoisted}
        for b in nc.m.functions[0].blocks:
            il = b.instructions
            kept = [i for i in il if id(i) not in ids]
            if len(kept) != len(il):
                il[:] = kept
        b0 = nc.m.functions[0].blocks[0].instructions
        b0[:] = hoisted + b0

    nc.compile()
    return nc


def get_module():
    global _cached_nc
    if _cached_nc is None:
        _cached_nc = _build_module()
    return _cached_nc


def _make_seg() -> np.ndarray:
    seg = np.zeros((P, BS), dtype=np.float32)
    seg[np.arange(P), np.arange(P) // RPS] = 1.0
    return seg


def make_in_maps(input: np.ndarray, target: np.ndarray) -> list[dict]:
    x = np.ascontiguousarray(input, dtype=np.float32)
    y = np.ascontiguousarray((target != 0).astype(np.uint8))
    seg8 = _make_seg().view(np.uint8)  # [P, BS*4]
    in_maps = []
    for c in range(NCORES):
        xs8 = x[c * BS : (c + 1) * BS].reshape(P, F).view(np.uint8)  # [P, F*4]
        ys8 = y[c * BS : (c + 1) * BS].reshape(P, F)  # [P, F]
        blob = np.concatenate([xs8, ys8, seg8], axis=1)  # [P, 704] u8
        in_maps.append({"blob": blob})
    return in_maps


def kernel(input: np.ndarray, target: np.ndarray) -> np.ndarray:
    input = np.asarray(input)
    target = np.asarray(target)
    assert input.shape == (B, L) and target.shape == (B, L)
    nc = get_module()
    in_maps = make_in_maps(input, target)
    res = run_bass_kernel_spmd(nc, in_maps, core_ids=list(range(NCORES)))
    losses = np.concatenate([np.asarray(r["out"]).reshape(BS) for r in res.results])
    return np.asarray(losses.sum(), dtype=np.float32)
